# revision 37
# baseline (speedup 1.0000x reference)
"""Trainium2 Bass kernel for nn_Attn_48206712930921.

softmax over s of energies[b,s] where energies[b,s] = outputs[b,s,:].v + c,
v = W^T @ weight_vec, c = weight_vec.b  (the [H,H] projection collapses to a
length-H dot product).  Rows s >= text_lens[b] softmax to exactly 0 (the
-1e10 fill underflows exp), so only the valid prefix of each sequence is
ever read: ~49.5% of the input.

Ragged schedule: each batch b occupies ceil(len_b/128) 128-row chunks;
whole batches are LPT-packed onto the 8 cores (near-perfect balance).  The
host packs each core's valid rows as fp16 in a [128, NCOL, H] layout so
every DMA descriptor is a long contiguous run per partition.  Each chunk's
energies are computed by a single fused DVE tensor_tensor_reduce
(x*v multiply + row reduction, fp16 2x mode), with the per-row mask/bias
(c for valid rows, -1e10 for pad rows) folded in via the accumulator init.
The per-batch softmax normalization runs on-device with host-supplied
chunk->batch membership matrices: per-chunk sums and per-batch sums are two
tiny TensorE matmuls, the reciprocal is scattered back to chunks by a third,
and a TensorE transpose puts probabilities in [chunk, row] layout for the
output DMA.  No max-subtraction is needed: energies are ~N(0,1) so exp is
safe in f32.
"""

import numpy as np
import ml_dtypes

import concourse.bacc as bacc
import concourse.bass as bass
import concourse.tile as tile
from concourse import mybir
from concourse.bass_utils import run_bass_kernel_spmd

B, S, H = 64, 2048, 1024
NCORES = 8
CHUNK = 128
NEG = -1.0e10
GROUP = 8            # chunks per DMA transfer (2 MiB bf16)

f32 = mybir.dt.float32
f16 = mybir.dt.bfloat16          # 16-bit stream dtype (device)
np16 = ml_dtypes.bfloat16        # matching numpy dtype (host)

# chunk compute path: "cdve" = fused custom-DVE tensor_tensor_reduce;
# "split" = DVE multiply + reduction split between ScalarE accum / DVE reduce
PATH = "split"
SCALAR_FRAC = 0.55               # split path: fraction of chunks on ScalarE
PE_BLOCKS = 0                    # 512-row blocks (4 chunks each) on TensorE
BROWS = 4 * CHUNK                # rows per PE block

_cached = {}


def _plan(lens):
    """LPT-pack whole batches onto cores by chunk count."""
    chunks = [(L + CHUNK - 1) // CHUNK for L in lens]
    order = sorted(range(B), key=lambda i: -chunks[i])
    bins = [[] for _ in range(NCORES)]
    loads = [0] * NCORES
    for i in order:
        k = loads.index(min(loads))
        bins[k].append(i)
        loads[k] += chunks[i]
    ncol = max(loads)
    maxb = max(len(bn) for bn in bins)
    assert ncol <= 128 and maxb <= 128
    return chunks, bins, ncol, maxb


def _groups(ncol):
    """(start, size) DMA groups.

    Small groups first so compute starts right away (pipeline ramp), then
    full-size groups, and a small remainder last to shrink the tail."""
    sizes = []
    for s in (2, 2, 4):
        if sum(sizes) + s <= ncol:
            sizes.append(s)
    while ncol - sum(sizes) >= GROUP:
        sizes.append(GROUP)
    if ncol - sum(sizes):
        sizes.append(ncol - sum(sizes))
    out = []
    c = 0
    for s in sizes:
        out.append((c, s))
        c += s
    return out


def _build(ncol, maxb):
    nc = bacc.Bacc("TRN2", target_bir_lowering=False, debug=False,
                   num_devices=NCORES)

    x = nc.dram_tensor("x", [CHUNK, ncol, H], f16, kind="ExternalInput")
    v = nc.dram_tensor("v", [H], f16, kind="ExternalInput")
    addv = nc.dram_tensor("addv", [CHUNK, ncol], f32, kind="ExternalInput")
    mm = nc.dram_tensor("mm", [ncol, maxb], f32, kind="ExternalInput")
    mmt = nc.dram_tensor("mmt", [maxb, ncol], f32, kind="ExternalInput")
    ident = nc.dram_tensor("ident", [CHUNK, CHUNK], f32, kind="ExternalInput")
    out = nc.dram_tensor("out", [ncol, CHUNK], f32, kind="ExternalOutput")

    with tile.TileContext(nc) as tc:
        with tc.tile_pool(name="singles", bufs=1) as singles, \
             tc.tile_pool(name="xp", bufs=3) as xp, \
             tc.tile_pool(name="prodp", bufs=3) as prodp, \
             tc.tile_pool(name="junkp", bufs=2) as junkp, \
             tc.tile_pool(name="gjunkp", bufs=3) as gjunkp, \
             tc.tile_pool(name="dumpp", bufs=2) as dumpp, \
             tc.tile_pool(name="sp", bufs=2) as sp, \
             tc.tile_pool(name="pp", bufs=2, space="PSUM") as pp, \
             tc.tile_pool(name="ptp", bufs=1, space="PSUM") as ptp:

            # v replicated across all 128 partitions via 0-stride DMA
            vb = singles.tile([CHUNK, H], f16)
            v_ap = v.ap()
            v_bcast = bass.AP(tensor=v_ap.tensor, offset=v_ap.offset,
                              ap=[[0, CHUNK]] + list(v_ap.ap))
            nc.gpsimd.dma_start(out=vb, in_=v_bcast)

            # issue the first two x transfers before the small constant
            # loads so the multiply pipeline starts as early as possible
            groups = _groups(ncol)
            xt_pre = {}
            for gi in (0, 1):
                if gi < len(groups):
                    c0, gsz = groups[gi]
                    xt = xp.tile([CHUNK, gsz, H], f16)
                    eng = nc.sync if gi % 2 == 0 else nc.gpsimd
                    eng.dma_start(out=xt, in_=x[:, c0:c0 + gsz, :])
                    xt_pre[gi] = xt

            addvt = singles.tile([CHUNK, ncol], f32)
            nc.gpsimd.dma_start(out=addvt, in_=addv[:, :])
            mmtl = singles.tile([ncol, maxb], f32)
            nc.gpsimd.dma_start(out=mmtl, in_=mm[:, :])
            mmttl = singles.tile([maxb, ncol], f32)
            nc.gpsimd.dma_start(out=mmttl, in_=mmt[:, :])
            identt = singles.tile([CHUNK, CHUNK], f32)
            nc.gpsimd.dma_start(out=identt, in_=ident[:, :])
            ones = singles.tile([CHUNK, 1], f32)
            nc.vector.memset(ones, 1.0)

            # energies, one column per chunk
            e = singles.tile([CHUNK, ncol], f32)
            e2 = singles.tile([CHUNK, ncol], f32)
            p = singles.tile([CHUNK, ncol], f32)
            cs_ps = pp.tile([ncol, 1], f32)

            vb_ap = vb[:, :]

            done = 0             # columns already masked+exp'd+chunk-summed
            for gi, (c0, gsz) in enumerate(groups):
                if gi in xt_pre:
                    xt = xt_pre[gi]
                else:
                    xt = xp.tile([CHUNK, gsz, H], f16)
                    eng = nc.sync if gi % 2 == 0 else nc.gpsimd
                    eng.dma_start(out=xt, in_=x[:, c0:c0 + gsz, :])
                if PATH == "cdve":
                    from concourse.dve_ops import TENSOR_TENSOR_REDUCE
                    for n in range(gsz):
                        c = c0 + n
                        junk = junkp.tile([CHUNK, H], f16)
                        # e[:,c] = addv[:,c] + sum_h x[:,c,h] * v[h]
                        nc.vector._custom_dve(
                            TENSOR_TENSOR_REDUCE, out=junk,
                            in0=xt[:, n, :], in1=vb,
                            s0=addvt[:, c:c + 1], s1=1.0,
                            accum_out=e[:, c:c + 1])
                else:
                    # one wide multiply per group, then per-chunk reductions
                    vb_rep = bass.AP(tensor=vb_ap.tensor, offset=vb_ap.offset,
                                     ap=[vb_ap.ap[0], [0, gsz], vb_ap.ap[1]])
                    prod = prodp.tile([CHUNK, gsz, H], f16)
                    nc.vector.tensor_mul(prod, xt, vb_rep)
                    # fold halves once on DVE (2x) so every reduction
                    # touches 512 elements instead of 1024
                    fold = gjunkp.tile([CHUNK, gsz, H // 2], f16)
                    nc.vector.tensor_add(fold, prod[:, :, :H // 2],
                                         prod[:, :, H // 2:])
                    last = gi >= len(groups) - 2
                    for n in range(gsz):
                        c = c0 + n
                        if (n % 2 == 0) if last else ((c % 20) < 2):
                            # DVE lane: tensor_scalar copy body + add-accum
                            junk = junkp.tile([CHUNK, H // 2], f16)
                            nc.vector.tensor_scalar(
                                out=junk, in0=fold[:, n, :],
                                scalar1=1.0, scalar2=0.0,
                                op0=mybir.AluOpType.mult,
                                op1=mybir.AluOpType.add,
                                accum_out=e[:, c:c + 1])
                        else:
                            # ScalarE lane: activation copy + accumulator
                            dump = dumpp.tile([CHUNK, H // 2], f16)
                            nc.scalar.activation(
                                out=dump, in_=fold[:, n, :],
                                func=mybir.ActivationFunctionType.Copy,
                                accum_out=e[:, c:c + 1])
                    if c0 + gsz in (32, 64) and c0 + gsz < ncol:
                        # columns [k0,k) done: mask+exp+chunk-sum them now so
                        # most of the softmax chain overlaps the loop tail
                        # (PSUM out base partition must be 0/32/64)
                        k = c0 + gsz
                        k0 = done
                        done = k
                        nc.vector.tensor_add(e2[:, k0:k], e[:, k0:k],
                                             addvt[:, k0:k])
                        nc.scalar.activation(
                            out=p[:, k0:k], in_=e2[:, k0:k],
                            func=mybir.ActivationFunctionType.Exp)
                        nc.tensor.matmul(cs_ps[k0:k], p[:, k0:k], ones,
                                         start=True, stop=True)

            # p = exp(e + addv); pad rows/chunks get -1e10 -> p = 0 exactly.
            # Columns below `done` were exp'd + chunk-summed inside the loop.
            sp48 = done
            nc.vector.tensor_add(e2[:, sp48:], e[:, sp48:], addvt[:, sp48:])
            nc.scalar.activation(out=p[:, sp48:], in_=e2[:, sp48:],
                                 func=mybir.ActivationFunctionType.Exp)
            if sp48 < ncol:
                nc.tensor.matmul(cs_ps[sp48:ncol], p[:, sp48:], ones,
                                 start=True, stop=True)

            # transpose early: depends only on p, runs off the chain
            pt_ps = ptp.tile([ncol, CHUNK], f32)
            nc.tensor.transpose(pt_ps, p, identt)

            cs = sp.tile([ncol, 1], f32)
            nc.scalar.copy(cs, cs_ps)
            # per-batch sums: bs[b] = sum_c mm[c, b] * cs[c]
            bs_ps = pp.tile([maxb, 1], f32)
            nc.tensor.matmul(bs_ps, mmtl, cs, start=True, stop=True)
            bs = sp.tile([maxb, 1], f32)
            # unused batch slots sum to 0; clamp so 1/0 can't poison matmuls
            nc.vector.tensor_scalar_max(bs, bs_ps, 1.0e-30)
            rb = sp.tile([maxb, 1], f32)
            nc.vector.reciprocal(rb, bs)
            # scatter 1/sum back to chunks: sc[c] = sum_b mmt[b, c] * rb[b]
            sc_ps = pp.tile([ncol, 1], f32)
            nc.tensor.matmul(sc_ps, mmttl, rb, start=True, stop=True)
            sc = sp.tile([ncol, 1], f32)
            nc.scalar.copy(sc, sc_ps)

            # scale each chunk row of the transposed probabilities by sc
            outt = sp.tile([ncol, CHUNK], f32)
            nc.vector.tensor_scalar_mul(outt, pt_ps, sc)
            nc.sync.dma_start(out=out[:, :], in_=outt)

    nc.compile()
    return nc


def _get(text_lens):
    lens = tuple(int(t) for t in np.asarray(text_lens))
    if lens not in _cached:
        chunks, bins, ncol, maxb = _plan(lens)
        nc = _build(ncol, maxb)
        _cached[lens] = (nc, chunks, bins, ncol, maxb)
    return _cached[lens]


def _in_maps(nc, chunks, bins, ncol, maxb, outputs, lens, W, b, weight_vec):
    W = np.asarray(W)
    bb = np.asarray(b)
    wv = np.asarray(weight_vec)
    v = (W.astype(np.float64).T @ wv.astype(np.float64)).astype(np16)
    c = np.float32(wv.astype(np.float64) @ bb.astype(np.float64))
    x16 = np.asarray(outputs).astype(np16)
    ident = np.eye(CHUNK, dtype=np.float32)

    maps = []
    for k in range(NCORES):
        xlin = np.zeros((ncol * CHUNK, H), np16)
        alin = np.full(ncol * CHUNK, NEG, np.float32)
        m = np.zeros((ncol, maxb), np.float32)
        c0 = 0
        for j, bi in enumerate(bins[k]):
            L = lens[bi]
            xlin[c0 * CHUNK:c0 * CHUNK + L] = x16[bi, :L]
            alin[c0 * CHUNK:c0 * CHUNK + L] = c
            m[c0:c0 + chunks[bi], j] = 1.0
            c0 += chunks[bi]
        xk = np.ascontiguousarray(
            xlin.reshape(ncol, CHUNK, H).transpose(1, 0, 2))
        ak = np.ascontiguousarray(alin.reshape(ncol, CHUNK).T)
        maps.append({"x": xk, "v": v, "addv": ak, "mm": m,
                     "mmt": np.ascontiguousarray(m.T), "ident": ident})
    return maps


def _gather(res, chunks, bins, lens):
    full = np.zeros((B, S), np.float32)
    for k in range(NCORES):
        flat = np.asarray(res.results[k]["out"]).reshape(-1)
        c0 = 0
        for bi in bins[k]:
            L = lens[bi]
            full[bi, :L] = flat[c0 * CHUNK:c0 * CHUNK + L]
            c0 += chunks[bi]
    return full


def kernel(outputs, text_lens, W, b, weight_vec):
    nc, chunks, bins, ncol, maxb = _get(text_lens)
    lens = [int(t) for t in np.asarray(text_lens)]
    maps = _in_maps(nc, chunks, bins, ncol, maxb, outputs, lens, W, b,
                    weight_vec)
    res = run_bass_kernel_spmd(nc, maps, list(range(NCORES)))
    return _gather(res, chunks, bins, lens)


def kernel_traced(outputs, text_lens, W, b, weight_vec, **trace_kwargs):
    """Like kernel() but profiles the run; returns (output, results)."""
    nc, chunks, bins, ncol, maxb = _get(text_lens)
    lens = [int(t) for t in np.asarray(text_lens)]
    maps = _in_maps(nc, chunks, bins, ncol, maxb, outputs, lens, W, b,
                    weight_vec)
    res = run_bass_kernel_spmd(nc, maps, list(range(NCORES)), trace=True,
                               **trace_kwargs)
    return _gather(res, chunks, bins, lens), res


# revision 39
# speedup vs baseline: 1.0282x; 1.0282x over previous
"""Trainium2 Bass kernel for nn_Attn_48206712930921.

softmax over s of energies[b,s] where energies[b,s] = outputs[b,s,:].v + c,
v = W^T @ weight_vec, c = weight_vec.b  (the [H,H] projection collapses to a
length-H dot product).  Rows s >= text_lens[b] softmax to exactly 0 (the
-1e10 fill underflows exp), so only the valid prefix of each sequence is
ever read: ~49.5% of the input.

Ragged schedule: each batch b occupies ceil(len_b/128) 128-row chunks;
whole batches are LPT-packed onto the 8 cores (near-perfect balance).  The
host packs each core's valid rows as bf16 in a [128, NCOL, H] layout so
every DMA descriptor is a long contiguous run per partition (~17.8 MB/core
instead of 64 MB).

Per DMA group the DVE does one wide bf16 multiply (2x packed mode) and one
half-fold add (prod[:512] + prod[512:]), halving what the reductions must
touch; per-partition row reductions run at 1x on every engine here, so the
512-element sums are split between ScalarE (activation Copy + accumulator,
~90% of chunks) and the DVE (tensor_scalar + add-accumulator), with the
last group's reductions alternated across both engines to shrink the tail.
The per-row mask/bias (c for valid rows, -1e10 for pad rows) is added
before a ScalarE exp, done in 32-column slabs as chunks complete so the
softmax chain overlaps the stream.  Per-batch normalization runs on-device
with host-supplied chunk->batch membership matrices: per-chunk and
per-batch sums are tiny TensorE matmuls, the reciprocal is scattered back
to chunks by a third, and a TensorE transpose puts probabilities in
[chunk, row] layout for the output DMA.  No max-subtraction is needed:
energies are ~N(0,1) so exp is safe in f32.
"""

import numpy as np
import ml_dtypes

import concourse.bacc as bacc
import concourse.bass as bass
import concourse.tile as tile
from concourse import mybir
from concourse.bass_utils import run_bass_kernel_spmd

B, S, H = 64, 2048, 1024
NCORES = 8
CHUNK = 128
NEG = -1.0e10
GROUP = 8            # chunks per DMA transfer (2 MiB bf16)

f32 = mybir.dt.float32
f16 = mybir.dt.bfloat16          # 16-bit stream dtype (device)
np16 = ml_dtypes.bfloat16        # matching numpy dtype (host)

# chunk compute path: "cdve" = fused custom-DVE tensor_tensor_reduce;
# "split" = DVE multiply + reduction split between ScalarE accum / DVE reduce
PATH = "split"
SCALAR_FRAC = 0.55               # split path: fraction of chunks on ScalarE
PE_BLOCKS = 0                    # 512-row blocks (4 chunks each) on TensorE
BROWS = 4 * CHUNK                # rows per PE block

_cached = {}


def _plan(lens):
    """LPT-pack whole batches onto cores by chunk count."""
    chunks = [(L + CHUNK - 1) // CHUNK for L in lens]
    order = sorted(range(B), key=lambda i: -chunks[i])
    bins = [[] for _ in range(NCORES)]
    loads = [0] * NCORES
    for i in order:
        k = loads.index(min(loads))
        bins[k].append(i)
        loads[k] += chunks[i]
    ncol = max(loads)
    maxb = max(len(bn) for bn in bins)
    assert ncol <= 128 and maxb <= 128
    return chunks, bins, ncol, maxb


def _groups(ncol):
    """(start, size) DMA groups.

    Small groups first so compute starts right away (pipeline ramp), then
    full-size groups, and a small remainder last to shrink the tail."""
    sizes = []
    for s in (2, 2, 4):
        if sum(sizes) + s <= ncol:
            sizes.append(s)
    while ncol - sum(sizes) >= GROUP:
        sizes.append(GROUP)
    if ncol - sum(sizes):
        sizes.append(ncol - sum(sizes))
    out = []
    c = 0
    for s in sizes:
        out.append((c, s))
        c += s
    return out


def _build(ncol, maxb):
    nc = bacc.Bacc("TRN2", target_bir_lowering=False, debug=False,
                   num_devices=NCORES)

    x = nc.dram_tensor("x", [CHUNK, ncol, H], f16, kind="ExternalInput")
    v = nc.dram_tensor("v", [H], f16, kind="ExternalInput")
    addv = nc.dram_tensor("addv", [CHUNK, ncol], f32, kind="ExternalInput")
    mm = nc.dram_tensor("mm", [ncol, maxb], f32, kind="ExternalInput")
    mmt = nc.dram_tensor("mmt", [maxb, ncol], f32, kind="ExternalInput")
    ident = nc.dram_tensor("ident", [CHUNK, CHUNK], f32, kind="ExternalInput")
    out = nc.dram_tensor("out", [ncol, CHUNK], f32, kind="ExternalOutput")

    with tile.TileContext(nc) as tc:
        with tc.tile_pool(name="singles", bufs=1) as singles, \
             tc.tile_pool(name="xp", bufs=3) as xp, \
             tc.tile_pool(name="prodp", bufs=3) as prodp, \
             tc.tile_pool(name="junkp", bufs=2) as junkp, \
             tc.tile_pool(name="gjunkp", bufs=3) as gjunkp, \
             tc.tile_pool(name="dumpp", bufs=2) as dumpp, \
             tc.tile_pool(name="sp", bufs=2) as sp, \
             tc.tile_pool(name="pp", bufs=2, space="PSUM") as pp, \
             tc.tile_pool(name="ptp", bufs=1, space="PSUM") as ptp:

            # v replicated across all 128 partitions via 0-stride DMA
            vb = singles.tile([CHUNK, H], f16)
            v_ap = v.ap()
            v_bcast = bass.AP(tensor=v_ap.tensor, offset=v_ap.offset,
                              ap=[[0, CHUNK]] + list(v_ap.ap))
            nc.gpsimd.dma_start(out=vb, in_=v_bcast)

            # issue the first two x transfers before the small constant
            # loads so the multiply pipeline starts as early as possible
            groups = _groups(ncol)
            xt_pre = {}
            for gi in (0, 1):
                if gi < len(groups):
                    c0, gsz = groups[gi]
                    xt = xp.tile([CHUNK, gsz, H], f16)
                    eng = nc.sync if gi % 2 == 0 else nc.gpsimd
                    eng.dma_start(out=xt, in_=x[:, c0:c0 + gsz, :])
                    xt_pre[gi] = xt

            addvt = singles.tile([CHUNK, ncol], f32)
            nc.gpsimd.dma_start(out=addvt, in_=addv[:, :])
            mmtl = singles.tile([ncol, maxb], f32)
            nc.gpsimd.dma_start(out=mmtl, in_=mm[:, :])
            mmttl = singles.tile([maxb, ncol], f32)
            nc.gpsimd.dma_start(out=mmttl, in_=mmt[:, :])
            identt = singles.tile([CHUNK, CHUNK], f32)
            nc.gpsimd.dma_start(out=identt, in_=ident[:, :])
            ones = singles.tile([CHUNK, 1], f32)
            nc.vector.memset(ones, 1.0)

            # energies, one column per chunk
            e = singles.tile([CHUNK, ncol], f32)
            e2 = singles.tile([CHUNK, ncol], f32)
            p = singles.tile([CHUNK, ncol], f32)
            cs_ps = pp.tile([ncol, 1], f32)

            vb_ap = vb[:, :]

            done = 0             # columns already masked+exp'd+chunk-summed
            for gi, (c0, gsz) in enumerate(groups):
                if gi in xt_pre:
                    xt = xt_pre[gi]
                else:
                    xt = xp.tile([CHUNK, gsz, H], f16)
                    eng = nc.sync if gi % 2 == 0 else nc.gpsimd
                    eng.dma_start(out=xt, in_=x[:, c0:c0 + gsz, :])
                if PATH == "cdve":
                    from concourse.dve_ops import TENSOR_TENSOR_REDUCE
                    for n in range(gsz):
                        c = c0 + n
                        junk = junkp.tile([CHUNK, H], f16)
                        # e[:,c] = addv[:,c] + sum_h x[:,c,h] * v[h]
                        nc.vector._custom_dve(
                            TENSOR_TENSOR_REDUCE, out=junk,
                            in0=xt[:, n, :], in1=vb,
                            s0=addvt[:, c:c + 1], s1=1.0,
                            accum_out=e[:, c:c + 1])
                else:
                    # one wide multiply per group, then per-chunk reductions
                    vb_rep = bass.AP(tensor=vb_ap.tensor, offset=vb_ap.offset,
                                     ap=[vb_ap.ap[0], [0, gsz], vb_ap.ap[1]])
                    prod = prodp.tile([CHUNK, gsz, H], f16)
                    nc.vector.tensor_mul(prod, xt, vb_rep)
                    # fold halves once on DVE (2x) so every reduction
                    # touches 512 elements instead of 1024
                    fold = gjunkp.tile([CHUNK, gsz, H // 2], f16)
                    nc.vector.tensor_add(fold, prod[:, :, :H // 2],
                                         prod[:, :, H // 2:])
                    last = gi == len(groups) - 1
                    for n in range(gsz):
                        c = c0 + n
                        if (n % 2 == 0) if last else ((c % 20) < 2):
                            # DVE lane: tensor_scalar copy body + add-accum
                            junk = junkp.tile([CHUNK, H // 2], f16)
                            nc.vector.tensor_scalar(
                                out=junk, in0=fold[:, n, :],
                                scalar1=1.0, scalar2=0.0,
                                op0=mybir.AluOpType.mult,
                                op1=mybir.AluOpType.add,
                                accum_out=e[:, c:c + 1])
                        else:
                            # ScalarE lane: activation copy + accumulator
                            dump = dumpp.tile([CHUNK, H // 2], f16)
                            nc.scalar.activation(
                                out=dump, in_=fold[:, n, :],
                                func=mybir.ActivationFunctionType.Copy,
                                accum_out=e[:, c:c + 1])
                    if c0 + gsz in (32, 64) and c0 + gsz < ncol:
                        # columns [k0,k) done: mask+exp+chunk-sum them now so
                        # most of the softmax chain overlaps the loop tail
                        # (PSUM out base partition must be 0/32/64)
                        k = c0 + gsz
                        k0 = done
                        done = k
                        nc.vector.tensor_add(e2[:, k0:k], e[:, k0:k],
                                             addvt[:, k0:k])
                        nc.scalar.activation(
                            out=p[:, k0:k], in_=e2[:, k0:k],
                            func=mybir.ActivationFunctionType.Exp)
                        nc.tensor.matmul(cs_ps[k0:k], p[:, k0:k], ones,
                                         start=True, stop=True)

            # p = exp(e + addv); pad rows/chunks get -1e10 -> p = 0 exactly.
            # Columns below `done` were exp'd + chunk-summed inside the loop.
            sp48 = done
            nc.vector.tensor_add(e2[:, sp48:], e[:, sp48:], addvt[:, sp48:])
            nc.scalar.activation(out=p[:, sp48:], in_=e2[:, sp48:],
                                 func=mybir.ActivationFunctionType.Exp)
            if sp48 < ncol:
                nc.tensor.matmul(cs_ps[sp48:ncol], p[:, sp48:], ones,
                                 start=True, stop=True)

            # transpose early: depends only on p, runs off the chain
            pt_ps = ptp.tile([ncol, CHUNK], f32)
            nc.tensor.transpose(pt_ps, p, identt)

            cs = sp.tile([ncol, 1], f32)
            nc.scalar.copy(cs, cs_ps)
            # per-batch sums: bs[b] = sum_c mm[c, b] * cs[c]
            bs_ps = pp.tile([maxb, 1], f32)
            nc.tensor.matmul(bs_ps, mmtl, cs, start=True, stop=True)
            bs = sp.tile([maxb, 1], f32)
            # unused batch slots sum to 0; clamp so 1/0 can't poison matmuls
            nc.vector.tensor_scalar_max(bs, bs_ps, 1.0e-30)
            rb = sp.tile([maxb, 1], f32)
            nc.vector.reciprocal(rb, bs)
            # scatter 1/sum back to chunks: sc[c] = sum_b mmt[b, c] * rb[b]
            sc_ps = pp.tile([ncol, 1], f32)
            nc.tensor.matmul(sc_ps, mmttl, rb, start=True, stop=True)
            sc = sp.tile([ncol, 1], f32)
            nc.scalar.copy(sc, sc_ps)

            # scale each chunk row of the transposed probabilities by sc
            outt = sp.tile([ncol, CHUNK], f32)
            nc.vector.tensor_scalar_mul(outt, pt_ps, sc)
            nc.sync.dma_start(out=out[:, :], in_=outt)

    nc.compile()
    return nc


def _get(text_lens):
    lens = tuple(int(t) for t in np.asarray(text_lens))
    if lens not in _cached:
        chunks, bins, ncol, maxb = _plan(lens)
        nc = _build(ncol, maxb)
        _cached[lens] = (nc, chunks, bins, ncol, maxb)
    return _cached[lens]


def _in_maps(nc, chunks, bins, ncol, maxb, outputs, lens, W, b, weight_vec):
    W = np.asarray(W)
    bb = np.asarray(b)
    wv = np.asarray(weight_vec)
    v = (W.astype(np.float64).T @ wv.astype(np.float64)).astype(np16)
    c = np.float32(wv.astype(np.float64) @ bb.astype(np.float64))
    x16 = np.asarray(outputs).astype(np16)
    ident = np.eye(CHUNK, dtype=np.float32)

    maps = []
    for k in range(NCORES):
        xlin = np.zeros((ncol * CHUNK, H), np16)
        alin = np.full(ncol * CHUNK, NEG, np.float32)
        m = np.zeros((ncol, maxb), np.float32)
        c0 = 0
        for j, bi in enumerate(bins[k]):
            L = lens[bi]
            xlin[c0 * CHUNK:c0 * CHUNK + L] = x16[bi, :L]
            alin[c0 * CHUNK:c0 * CHUNK + L] = c
            m[c0:c0 + chunks[bi], j] = 1.0
            c0 += chunks[bi]
        xk = np.ascontiguousarray(
            xlin.reshape(ncol, CHUNK, H).transpose(1, 0, 2))
        ak = np.ascontiguousarray(alin.reshape(ncol, CHUNK).T)
        maps.append({"x": xk, "v": v, "addv": ak, "mm": m,
                     "mmt": np.ascontiguousarray(m.T), "ident": ident})
    return maps


def _gather(res, chunks, bins, lens):
    full = np.zeros((B, S), np.float32)
    for k in range(NCORES):
        flat = np.asarray(res.results[k]["out"]).reshape(-1)
        c0 = 0
        for bi in bins[k]:
            L = lens[bi]
            full[bi, :L] = flat[c0 * CHUNK:c0 * CHUNK + L]
            c0 += chunks[bi]
    return full


def kernel(outputs, text_lens, W, b, weight_vec):
    nc, chunks, bins, ncol, maxb = _get(text_lens)
    lens = [int(t) for t in np.asarray(text_lens)]
    maps = _in_maps(nc, chunks, bins, ncol, maxb, outputs, lens, W, b,
                    weight_vec)
    res = run_bass_kernel_spmd(nc, maps, list(range(NCORES)))
    return _gather(res, chunks, bins, lens)


def kernel_traced(outputs, text_lens, W, b, weight_vec, **trace_kwargs):
    """Like kernel() but profiles the run; returns (output, results)."""
    nc, chunks, bins, ncol, maxb = _get(text_lens)
    lens = [int(t) for t in np.asarray(text_lens)]
    maps = _in_maps(nc, chunks, bins, ncol, maxb, outputs, lens, W, b,
                    weight_vec)
    res = run_bass_kernel_spmd(nc, maps, list(range(NCORES)), trace=True,
                               **trace_kwargs)
    return _gather(res, chunks, bins, lens), res
